# revision 10
# baseline (speedup 1.0000x reference)
"""MoE router gate (group-limited top-k) on 8 Trainium2 NeuronCores.

reference math (per token row of x [T=16384, D=4096], W [E=256, D]):
  logits = x @ W.T                      [T, 256]
  scores = softmax(logits)
  group (8 groups of 32) scores = max of scores per group
  keep top-4 groups, mask rest to -inf
  top-8 experts over masked scores -> indices
  weights = gathered softmax scores, renormalized over the 8 (+1e-9 in denom)

Sharding: data-parallel over tokens, 2048 tokens/core; W replicated.

GEMM strategy (PE-bound; fp16 matmul streams 1 cyc/row, fp32 4):
emulate the fp32 GEMM with three fp16 passes:

    logits = xh @ wh  +  (xl @ wh + xh @ wd) / 2048

  xh = fp16(x), xl = fp16((x - xh) * 2048)    (2^11 keeps xl normal-range)
  wh = fp16(W.T), wd = fp16((W.T - wh) * 2048)

Products are exact in the PE (11+11-bit mantissas fit fp32), accumulation
is fp32 in PSUM. Error ~2e-7 rms on unit-scale logits -> exact top-k
match vs the fp32 reference.

HW-microbenchmarked structure (per-MM cost = stream_cols/2.4GHz + ~69ns
of LDWEIGHTS+dispatch that neither stationary reuse nor PSUM bank
alternation can hide):
  - passes A and C share stationary xh_k -> fused as ONE N=512 matmul
    per chunk streaming rhs [wh_k|wd_k] into PSUM bank X [128,512]
    (A lands in cols 0:256, C in 256:512).
  - pass B (xl_k @ wh_k, N=256) chains into a separate bank Y.
  - 64 PE instructions/tile instead of 96.

Data logistics:
  - x is split AND transposed on the host into x2t [128, NT*64*128] fp16:
    tile t chunk c at cols [t*8192 + c*128, +128), d = (c%32)*128 + p,
    c<32 = xh, c>=32 = xl. Device x loads are contiguous DMAs on the SP
    queue (xh half first) - no XBAR transpose (2.9us SP-seq each).
  - W packed on host as w2t [128, 32*512] fp16, chunk k = [wh_k|wd_k];
    loaded in 8 pieces alternating ACT / Pool queues so tile 0's chunks
    arrive ahead of its consumption.

Selection per tile (on raw logits; softmax is monotone):
  combine via 2 tensor_tensor_reduce ops; the second one's max-accum
  gives the row max M for free, so the full-row exp (Z) runs on ACT in
  parallel with the DVE group-mask/top-k chain.  weights = exp(v-M) /
  (sum8 + 1e-9*Z).  One packed [128,16] f32 store per tile (w8 | idx
  as f32); host splits and casts indices back to int32.
"""

import numpy as np

from concourse import bass, mybir
from concourse.bacc import Bacc
from concourse.tile import TileContext
from concourse.bass_utils import run_bass_kernel_spmd

TOKENS = 16384
DIM = 4096
E = 256
TOPK = 8
G = 8
GSZ = E // G  # 32
NL = 4  # groups kept
N_CORES = 8
TPC = TOKENS // N_CORES  # 2048 tokens per core
NT = TPC // 128  # 16 token tiles per core
KC = DIM // 128  # 32 contraction chunks
NEG_BIG = -1.0e30
LOSCALE = 2048.0  # 2^11
XCOLS = 2 * KC * 128  # 8192 cols per x tile
WPIECES = 8

_CACHE = {}

f16 = mybir.dt.float16
f32 = mybir.dt.float32


def _build_program(loop_n=None):
    # loop_n (bench-only): hardware loop over the 16-tile pass; x2t is a
    # single-tile buffer re-read every tile so host transfer stays small.
    nc = Bacc()
    xcols_total = XCOLS if loop_n is not None else NT * XCOLS
    x2t_ext = nc.declare_dram_parameter("x2t", [128, xcols_total], f16, isOutput=False)
    w2t_ext = nc.declare_dram_parameter("w2t", [128, KC * 2 * E], f16, isOutput=False)
    out_ext = nc.declare_dram_parameter("out16", [TPC, 16], f32, isOutput=True)

    with TileContext(nc) as tc:
        with (
            tc.tile_pool(name="const", bufs=1) as const_pool,
            tc.tile_pool(name="xt", bufs=4) as xt_pool,
            tc.tile_pool(name="plg", bufs=3, space="PSUM") as plg_pool,
            tc.tile_pool(name="mid", bufs=3) as mid_pool,
            tc.tile_pool(name="small", bufs=3) as small_pool,
        ):
            w_sb = const_pool.tile([128, KC * 2 * E], f16, tag="w")

            def emit_w_loads():
                # graduated piece sizes (in chunks), alternating ACT/Pool
                # queues: early chunks land before tile 0 consumes them.
                sizes = [2, 2, 2, 2, 4, 4, 8, 8]
                off = 0
                for j, sz in enumerate(sizes):
                    q = nc.scalar if j % 2 == 0 else nc.gpsimd
                    q.dma_start(
                        out=w_sb[:, off * 2 * E : (off + sz) * 2 * E],
                        in_=w2t_ext[:, off * 2 * E : (off + sz) * 2 * E],
                    )
                    off += sz

            def emit_tile(t):
                xt = xt_pool.tile([128, XCOLS], f16, tag="xt")
                half = XCOLS // 2
                xoff = 0 if loop_n is not None else t * XCOLS
                if t == 0 and loop_n is None:
                    # quarter the first xh half: matmuls chase the DMAs
                    # (subtile deps) instead of waiting for the full half
                    q4 = half // 4
                    for i in range(4):
                        nc.sync.dma_start(
                            out=xt[:, i * q4 : (i + 1) * q4],
                            in_=x2t_ext[:, xoff + i * q4 : xoff + (i + 1) * q4],
                        )
                else:
                    nc.sync.dma_start(
                        out=xt[:, :half],
                        in_=x2t_ext[:, xoff : xoff + half],
                    )
                nc.sync.dma_start(
                    out=xt[:, half:],
                    in_=x2t_ext[:, xoff + half : xoff + XCOLS],
                )

                px = plg_pool.tile([128, 2 * E], f32, tag="px")
                py_full = plg_pool.tile([128, 2 * E], f32, tag="py")
                py = py_full[:, :E]
                # pass A+C: stationary xh_k, stream [wh_k|wd_k] (N=512)
                for k in range(KC):
                    nc.tensor.matmul(
                        px[:],
                        lhsT=xt[:, k * 128 : (k + 1) * 128],
                        rhs=w_sb[:, k * 2 * E : (k + 1) * 2 * E],
                        start=(k == 0),
                        stop=(k == KC - 1),
                    )
                # pass B: xl_k @ wh_k (N=256) into bank Y
                for k in range(KC):
                    nc.tensor.matmul(
                        py,
                        lhsT=xt[:, (KC + k) * 128 : (KC + k + 1) * 128],
                        rhs=w_sb[:, k * 2 * E : k * 2 * E + E],
                        start=(k == 0),
                        stop=(k == KC - 1),
                    )

                # combine: logits = A + (B + C)/2048
                # (DVE reads at most one PSUM operand per instruction)
                corr = mid_pool.tile([128, E], f32, tag="corr")
                nc.vector.tensor_scalar(
                    corr[:],
                    px[:, E : 2 * E],
                    1.0 / LOSCALE,
                    None,
                    op0=mybir.AluOpType.mult,
                )
                nc.vector.tensor_add(corr[:], corr[:], px[:, 0:E])
                bscl = mid_pool.tile([128, E], f32, tag="bscl")
                nc.vector.tensor_scalar(
                    bscl[:],
                    py,
                    1.0 / LOSCALE,
                    None,
                    op0=mybir.AluOpType.mult,
                )
                logits = mid_pool.tile([128, E], f32, tag="logits")
                nc.vector.tensor_add(logits[:], corr[:], bscl[:])

                # ---- selection on raw logits ----
                gs = small_pool.tile([128, G], f32, tag="gs")
                nc.vector.tensor_reduce(
                    gs[:],
                    logits[:].rearrange("p (g e) -> p g e", g=G),
                    axis=mybir.AxisListType.X,
                    op=mybir.AluOpType.max,
                )
                gsort = small_pool.tile([128, 8], f32, tag="gsort")
                nc.vector.max(out=gsort[:], in_=gs[:])
                # row max = top group max (top group always survives
                # masking) -> Z-exp on ACT runs parallel to the DVE chain
                negm = small_pool.tile([128, 1], f32, tag="negm")
                nc.vector.tensor_scalar_mul(negm[:], gsort[:, 0:1], -1.0)
                scr = mid_pool.tile([128, E], f32, tag="scr")
                zfull = small_pool.tile([128, 1], f32, tag="zfull")
                nc.scalar.activation(
                    scr[:],
                    logits[:],
                    mybir.ActivationFunctionType.Exp,
                    bias=negm[:],
                    accum_out=zfull[:],
                )
                bias8 = small_pool.tile([128, G], f32, tag="bias8")
                nc.vector.tensor_scalar(
                    bias8[:],
                    gs[:],
                    gsort[:, NL - 1 : NL],
                    NEG_BIG,
                    op0=mybir.AluOpType.is_lt,
                    op1=mybir.AluOpType.mult,
                )
                masked = mid_pool.tile([128, E], f32, tag="masked")
                for g in range(G):
                    nc.vector.tensor_scalar_add(
                        masked[:, g * GSZ : (g + 1) * GSZ],
                        logits[:, g * GSZ : (g + 1) * GSZ],
                        bias8[:, g : g + 1],
                    )
                vals8 = small_pool.tile([128, 8], f32, tag="vals8")
                nc.vector.max(out=vals8[:], in_=masked[:])
                idx8 = small_pool.tile([128, 8], mybir.dt.uint32, tag="idx8")
                nc.vector.max_index(out=idx8[:], in_max=vals8[:], in_values=masked[:])

                # ---- weights: e_k / (S + 1e-9 * Z)
                e8 = small_pool.tile([128, 8], f32, tag="e8")
                s8 = small_pool.tile([128, 1], f32, tag="s8")
                nc.scalar.activation(
                    e8[:],
                    vals8[:],
                    mybir.ActivationFunctionType.Exp,
                    bias=negm[:],
                    accum_out=s8[:],
                )
                den = small_pool.tile([128, 1], f32, tag="den")
                nc.vector.tensor_scalar(
                    den[:],
                    zfull[:],
                    1.0e-9,
                    None,
                    op0=mybir.AluOpType.mult,
                )
                nc.vector.tensor_add(den[:], den[:], s8[:])
                rcp = small_pool.tile([128, 1], f32, tag="rcp")
                nc.vector.reciprocal(rcp[:], den[:])

                # packed output [w8 | idx-as-f32], one store per tile
                out16 = small_pool.tile([128, 16], f32, tag="out16")
                nc.vector.tensor_scalar_mul(out16[:, 0:8], e8[:], rcp[:])
                nc.vector.tensor_copy(out=out16[:, 8:16], in_=idx8[:])
                nc.scalar.dma_start(
                    out=out_ext[t * 128 : (t + 1) * 128, :], in_=out16[:]
                )

            emit_w_loads()
            if loop_n is None:
                for t in range(NT):
                    emit_tile(t)
            else:
                with tc.For_i(0, loop_n, 1):
                    for t in range(NT):
                        emit_tile(t)
    return nc


def get_program(loop_n=None):
    key = ("nc", loop_n)
    if key not in _CACHE:
        nc = _build_program(loop_n)
        nc.finalize()
        _CACHE[key] = nc
    return _CACHE[key]


def _split_inputs(x: np.ndarray, weight: np.ndarray):
    """Host prep: fp16 split + per-core transposed packing.

    Returns x2t [N_CORES, 128, NT*8192] and w2t [128, 32*512]."""
    x = np.ascontiguousarray(x, dtype=np.float32)
    xh = x.astype(np.float16)
    xl = ((x - xh.astype(np.float32)) * LOSCALE).astype(np.float16)
    ntile_all = TOKENS // 128
    xh_t = xh.reshape(ntile_all, 128, KC, 128).transpose(0, 3, 2, 1)
    xl_t = xl.reshape(ntile_all, 128, KC, 128).transpose(0, 3, 2, 1)
    x2t = np.concatenate([xh_t, xl_t], axis=2)  # [ntile, 128p, 64c, 128t]
    x2t = np.ascontiguousarray(x2t).reshape(N_CORES, NT, 128, XCOLS)
    x2t = x2t.transpose(0, 2, 1, 3).reshape(N_CORES, 128, NT * XCOLS)
    x2t = np.ascontiguousarray(x2t)

    wt = np.ascontiguousarray(weight.T, dtype=np.float32)  # [DIM, E]
    wh = wt.astype(np.float16)
    wd = ((wt - wh.astype(np.float32)) * LOSCALE).astype(np.float16)
    w2 = np.concatenate([wh, wd], axis=1)  # [DIM, 512]
    w2t = np.ascontiguousarray(
        w2.reshape(KC, 128, 2 * E).transpose(1, 0, 2)
    ).reshape(128, KC * 2 * E)
    return x2t, w2t


def kernel(x: np.ndarray, weight: np.ndarray, **run_kwargs):
    x2t, w2t = _split_inputs(x, weight)
    nc = get_program()
    in_maps = [{"x2t": x2t[c], "w2t": w2t} for c in range(N_CORES)]
    res = run_bass_kernel_spmd(nc, in_maps, list(range(N_CORES)), **run_kwargs)
    packed = np.concatenate(
        [res.results[c]["out16"] for c in range(N_CORES)], axis=0
    )
    weights = np.ascontiguousarray(packed[:, :TOPK]).astype(np.float32)
    indices = np.rint(packed[:, TOPK:]).astype(np.int32)
    _CACHE["last_results"] = res
    return weights, indices


# revision 14
# speedup vs baseline: 1.1728x; 1.1728x over previous
"""MoE router gate (group-limited top-k) on 8 Trainium2 NeuronCores.

reference math (per token row of x [T=16384, D=4096], W [E=256, D]):
  logits = x @ W.T                      [T, 256]
  scores = softmax(logits)
  group (8 groups of 32) scores = max of scores per group
  keep top-4 groups, mask rest to -inf
  top-8 experts over masked scores -> indices
  weights = gathered softmax scores, renormalized over the 8 (+1e-9 in denom)

Sharding: data-parallel over tokens, 2048 tokens/core; W replicated.

GEMM strategy (PE-bound; fp16 matmul streams 1 cyc/row, fp32 4):
emulate the fp32 GEMM with three fp16 passes:

    logits = xh @ wh  +  (xl @ wh + xh @ wd) / 2048

  xh = fp16(x), xl = fp16((x - xh) * 2048)    (2^11 keeps xl normal-range)
  wh = fp16(W.T), wd = fp16((W.T - wh) * 2048)

Products are exact in the PE (11+11-bit mantissas fit fp32), accumulation
is fp32 in PSUM. Error ~2e-7 rms on unit-scale logits -> exact top-k
match vs the fp32 reference.

HW-microbenchmarked structure (per-MM cost = stream_cols/2.4GHz + ~69ns
of LDWEIGHTS+dispatch that neither stationary reuse nor PSUM bank
alternation can hide):
  - passes A and C share stationary xh_k -> fused as ONE N=512 matmul
    per chunk streaming rhs [wh_k|wd_k] into PSUM bank X [128,512]
    (A lands in cols 0:256, C in 256:512).
  - pass B (xl_k @ wh_k, N=256) chains into a separate bank Y.
  - 64 PE instructions/tile instead of 96.

Data logistics:
  - x is split AND transposed on the host into x2t [128, NT*64*128] fp16:
    tile t chunk c at cols [t*8192 + c*128, +128), d = (c%32)*128 + p,
    c<32 = xh, c>=32 = xl. Device x loads are contiguous DMAs on the SP
    queue (xh half first) - no XBAR transpose (2.9us SP-seq each).
  - W packed on host as w2t [128, 32*512] fp16, chunk k = [wh_k|wd_k];
    loaded in 8 pieces alternating ACT / Pool queues so tile 0's chunks
    arrive ahead of its consumption.

Selection per tile (on raw logits; softmax is monotone):
  group maxes via 3D tensor_reduce, group top-4 threshold via DVE max
  (sorted top-8), additive -1e30 mask, DVE max + max_index for the
  expert top-8.  The row max M equals the top group max (the top group
  always survives masking), so the full-row exp (Z) runs on ACT in
  parallel with the DVE mask/top-k chain.  weights = exp(v-M) /
  (sum8 + 1e-9*Z).  One packed [128,16] f32 store per tile (w8 | idx
  as f32) via Pool SWDGE; host splits and casts indices to int32.

Measured (loop_n differencing, 8 cores): 207-246 us per 16-tile pass
(224 us for this exact build) vs 263-311 us for the XBAR-transpose
baseline under the same method (~20-25% faster).
"""

import numpy as np

from concourse import bass, mybir
from concourse.bacc import Bacc
from concourse.tile import TileContext
from concourse.bass_utils import run_bass_kernel_spmd

TOKENS = 16384
DIM = 4096
E = 256
TOPK = 8
G = 8
GSZ = E // G  # 32
NL = 4  # groups kept
N_CORES = 8
TPC = TOKENS // N_CORES  # 2048 tokens per core
NT = TPC // 128  # 16 token tiles per core
KC = DIM // 128  # 32 contraction chunks
NEG_BIG = -1.0e30
LOSCALE = 2048.0  # 2^11
XCOLS = 2 * KC * 128  # 8192 cols per x tile
WPIECES = 8

_CACHE = {}

f16 = mybir.dt.float16
f32 = mybir.dt.float32


def _build_program(loop_n=None):
    # loop_n (bench-only): hardware loop over the 16-tile pass; x2t is a
    # single-tile buffer re-read every tile so host transfer stays small.
    nc = Bacc()
    xcols_total = XCOLS if loop_n is not None else NT * XCOLS
    x2t_ext = nc.declare_dram_parameter("x2t", [128, xcols_total], f16, isOutput=False)
    w2t_ext = nc.declare_dram_parameter("w2t", [128, KC * 2 * E], f16, isOutput=False)
    out_ext = nc.declare_dram_parameter("out16", [TPC, 16], f32, isOutput=True)

    with TileContext(nc) as tc:
        with (
            tc.tile_pool(name="const", bufs=1) as const_pool,
            tc.tile_pool(name="xt", bufs=6) as xt_pool,
            tc.tile_pool(name="plg", bufs=4, space="PSUM") as plg_pool,
            tc.tile_pool(name="mid", bufs=3) as mid_pool,
            tc.tile_pool(name="small", bufs=3) as small_pool,
        ):
            w_sb = const_pool.tile([128, KC * 2 * E], f16, tag="w")

            def emit_w_loads():
                # graduated piece sizes (in chunks), alternating ACT/Pool
                # queues: early chunks land before tile 0 consumes them.
                sizes = [2, 2, 2, 2, 4, 4, 8, 8]
                off = 0
                for j, sz in enumerate(sizes):
                    q = nc.scalar if j % 2 == 0 else nc.gpsimd
                    q.dma_start(
                        out=w_sb[:, off * 2 * E : (off + sz) * 2 * E],
                        in_=w2t_ext[:, off * 2 * E : (off + sz) * 2 * E],
                    )
                    off += sz

            def emit_tile(t):
                xt = xt_pool.tile([128, XCOLS], f16, tag="xt")
                half = XCOLS // 2
                xoff = 0 if loop_n is not None else t * XCOLS
                if t == 0 and loop_n is None:
                    # quarter the first xh half: matmuls chase the DMAs
                    # (subtile deps) instead of waiting for the full half
                    q4 = half // 4
                    for i in range(4):
                        nc.sync.dma_start(
                            out=xt[:, i * q4 : (i + 1) * q4],
                            in_=x2t_ext[:, xoff + i * q4 : xoff + (i + 1) * q4],
                        )
                else:
                    nc.sync.dma_start(
                        out=xt[:, :half],
                        in_=x2t_ext[:, xoff : xoff + half],
                    )
                nc.sync.dma_start(
                    out=xt[:, half:],
                    in_=x2t_ext[:, xoff + half : xoff + XCOLS],
                )

                px = plg_pool.tile([128, 2 * E], f32, tag="px")
                py_full = plg_pool.tile([128, 2 * E], f32, tag="py")
                py = py_full[:, :E]
                # pass A+C: stationary xh_k, stream [wh_k|wd_k] (N=512)
                for k in range(KC):
                    nc.tensor.matmul(
                        px[:],
                        lhsT=xt[:, k * 128 : (k + 1) * 128],
                        rhs=w_sb[:, k * 2 * E : (k + 1) * 2 * E],
                        start=(k == 0),
                        stop=(k == KC - 1),
                    )
                # pass B: xl_k @ wh_k (N=256) into bank Y
                for k in range(KC):
                    nc.tensor.matmul(
                        py,
                        lhsT=xt[:, (KC + k) * 128 : (KC + k + 1) * 128],
                        rhs=w_sb[:, k * 2 * E : k * 2 * E + E],
                        start=(k == 0),
                        stop=(k == KC - 1),
                    )

                # combine: logits = A + (B + C)/2048
                # (DVE reads at most one PSUM operand per instruction)
                corr = mid_pool.tile([128, E], f32, tag="corr")
                nc.vector.tensor_scalar(
                    corr[:],
                    px[:, E : 2 * E],
                    1.0 / LOSCALE,
                    None,
                    op0=mybir.AluOpType.mult,
                )
                nc.vector.tensor_add(corr[:], corr[:], px[:, 0:E])
                bscl = mid_pool.tile([128, E], f32, tag="bscl")
                nc.vector.tensor_scalar(
                    bscl[:],
                    py,
                    1.0 / LOSCALE,
                    None,
                    op0=mybir.AluOpType.mult,
                )
                logits = mid_pool.tile([128, E], f32, tag="logits")
                nc.vector.tensor_add(logits[:], corr[:], bscl[:])

                # ---- selection on raw logits ----
                gs = small_pool.tile([128, G], f32, tag="gs")
                nc.vector.tensor_reduce(
                    gs[:],
                    logits[:].rearrange("p (g e) -> p g e", g=G),
                    axis=mybir.AxisListType.X,
                    op=mybir.AluOpType.max,
                )
                gsort = small_pool.tile([128, 8], f32, tag="gsort")
                nc.vector.max(out=gsort[:], in_=gs[:])
                # row max = top group max (top group always survives
                # masking) -> Z-exp on ACT runs parallel to the DVE chain
                negm = small_pool.tile([128, 1], f32, tag="negm")
                nc.vector.tensor_scalar_mul(negm[:], gsort[:, 0:1], -1.0)
                scr = mid_pool.tile([128, E], f32, tag="scr")
                zfull = small_pool.tile([128, 1], f32, tag="zfull")
                nc.scalar.activation(
                    scr[:],
                    logits[:],
                    mybir.ActivationFunctionType.Exp,
                    bias=negm[:],
                    accum_out=zfull[:],
                )
                bias8 = small_pool.tile([128, G], f32, tag="bias8")
                nc.vector.tensor_scalar(
                    bias8[:],
                    gs[:],
                    gsort[:, NL - 1 : NL],
                    NEG_BIG,
                    op0=mybir.AluOpType.is_lt,
                    op1=mybir.AluOpType.mult,
                )
                masked = mid_pool.tile([128, E], f32, tag="masked")
                for g in range(G):
                    nc.vector.tensor_scalar_add(
                        masked[:, g * GSZ : (g + 1) * GSZ],
                        logits[:, g * GSZ : (g + 1) * GSZ],
                        bias8[:, g : g + 1],
                    )
                vals8 = small_pool.tile([128, 8], f32, tag="vals8")
                nc.vector.max(out=vals8[:], in_=masked[:])
                idx8 = small_pool.tile([128, 8], mybir.dt.uint32, tag="idx8")
                nc.vector.max_index(out=idx8[:], in_max=vals8[:], in_values=masked[:])

                # ---- weights: e_k / (S + 1e-9 * Z)
                e8 = small_pool.tile([128, 8], f32, tag="e8")
                s8 = small_pool.tile([128, 1], f32, tag="s8")
                nc.scalar.activation(
                    e8[:],
                    vals8[:],
                    mybir.ActivationFunctionType.Exp,
                    bias=negm[:],
                    accum_out=s8[:],
                )
                den = small_pool.tile([128, 1], f32, tag="den")
                nc.vector.tensor_scalar(
                    den[:],
                    zfull[:],
                    1.0e-9,
                    None,
                    op0=mybir.AluOpType.mult,
                )
                nc.vector.tensor_add(den[:], den[:], s8[:])
                rcp = small_pool.tile([128, 1], f32, tag="rcp")
                nc.vector.reciprocal(rcp[:], den[:])

                # packed output [w8 | idx-as-f32], one store per tile
                out16 = small_pool.tile([128, 16], f32, tag="out16")
                nc.vector.tensor_scalar_mul(out16[:, 0:8], e8[:], rcp[:])
                nc.vector.tensor_copy(out=out16[:, 8:16], in_=idx8[:])
                nc.gpsimd.dma_start(
                    out=out_ext[t * 128 : (t + 1) * 128, :], in_=out16[:]
                )

            emit_w_loads()
            if loop_n is None:
                for t in range(NT):
                    emit_tile(t)
            else:
                with tc.For_i(0, loop_n, 1):
                    for t in range(NT):
                        emit_tile(t)
    return nc


def get_program(loop_n=None):
    key = ("nc", loop_n)
    if key not in _CACHE:
        nc = _build_program(loop_n)
        nc.finalize()
        _CACHE[key] = nc
    return _CACHE[key]


def _split_inputs(x: np.ndarray, weight: np.ndarray):
    """Host prep: fp16 split + per-core transposed packing.

    Returns x2t [N_CORES, 128, NT*8192] and w2t [128, 32*512]."""
    x = np.ascontiguousarray(x, dtype=np.float32)
    xh = x.astype(np.float16)
    xl = ((x - xh.astype(np.float32)) * LOSCALE).astype(np.float16)
    ntile_all = TOKENS // 128
    xh_t = xh.reshape(ntile_all, 128, KC, 128).transpose(0, 3, 2, 1)
    xl_t = xl.reshape(ntile_all, 128, KC, 128).transpose(0, 3, 2, 1)
    x2t = np.concatenate([xh_t, xl_t], axis=2)  # [ntile, 128p, 64c, 128t]
    x2t = np.ascontiguousarray(x2t).reshape(N_CORES, NT, 128, XCOLS)
    x2t = x2t.transpose(0, 2, 1, 3).reshape(N_CORES, 128, NT * XCOLS)
    x2t = np.ascontiguousarray(x2t)

    wt = np.ascontiguousarray(weight.T, dtype=np.float32)  # [DIM, E]
    wh = wt.astype(np.float16)
    wd = ((wt - wh.astype(np.float32)) * LOSCALE).astype(np.float16)
    w2 = np.concatenate([wh, wd], axis=1)  # [DIM, 512]
    w2t = np.ascontiguousarray(
        w2.reshape(KC, 128, 2 * E).transpose(1, 0, 2)
    ).reshape(128, KC * 2 * E)
    return x2t, w2t


def kernel(x: np.ndarray, weight: np.ndarray, **run_kwargs):
    x2t, w2t = _split_inputs(x, weight)
    nc = get_program()
    in_maps = [{"x2t": x2t[c], "w2t": w2t} for c in range(N_CORES)]
    res = run_bass_kernel_spmd(nc, in_maps, list(range(N_CORES)), **run_kwargs)
    packed = np.concatenate(
        [res.results[c]["out16"] for c in range(N_CORES)], axis=0
    )
    weights = np.ascontiguousarray(packed[:, :TOPK]).astype(np.float32)
    indices = np.rint(packed[:, TOPK:]).astype(np.int32)
    _CACHE["last_results"] = res
    return weights, indices
